# revision 53
# baseline (speedup 1.0000x reference)
"""Neighborhood attention (NATTEN-style 7x7 window, 4 heads, rpb) on 8
Trainium2 NeuronCores via Bass/Tile.

Sharding: 8 cores = 4 batches x 2 row-slabs (rows 0-27 / 28-55). Each core
gets a 34-row halo'd slab (3 zero pad rows at one edge so both slab variants
share one SPMD program), host-transposed to xT [C=128, 1904 tokens] bf16.

Per-core dataflow:
  qT = (x Wq*scale)^T  tile-major     (PE per 14x8 query tile; heads stacked
                                       on partitions, ACT escape w/ q-bias)
  kT = (x Wk)^T  row-major            (PE + ACT cast escape)
  v_nat = x Wv  [token, C] -> DRAM    (PE + DVE cast, one staging store)
  per 14x8 query tile (112 queries, padded 20x16 key band = 320 keys):
    scores[112,320] x4 heads          (PE, one matmul per head packed into
                                       32-row groups via tile_position ->
                                       4 PSUM banks)
    expS = exp(scores)                (ACT, 2 heads per op, PSUM->SBUF bf16)
    P = expS * expB                   (DVE; expB = host table exp(bias) *
                                       window mask, 0 outside the window ->
                                       masking and bias in one multiply)
    P^T                               (one chunked XBAR DMA-transpose
                                       [128,384] -> 3x[128,128])
    av = P^T.T @ [V|1] per 128-chunk  (PE; v_band gathered from v_nat DRAM
                                       with an interleaved ones column ->
                                       col 32 = softmax denominator)
    attn = av[:, :32] * recip(av[:,32])  (DVE)
  attnT = DMA-transpose(attn [112,128])
  out = attnT.T @ Wp -> f32 -> HBM    (PE + ACT escape, single store)

b_qkv: k-bias is softmax-invariant (dropped); v-bias folds into the output
bias host-side (sum(attn)=1); q-bias applied in the qT escape. Softmax skips
the max-subtraction (|logits| <~ 30 here, exp is safe in f32->bf16).
"""

import numpy as np
import ml_dtypes

KS = 7
NS = KS // 2
NH = 4
DIM = 128
HD = DIM // NH
SCALE = HD ** -0.5

B, H, W = 4, 56, 56
SLAB = 28            # query rows per core
HALO_ROWS = 34       # 28 + 3 halo + 3 uniform pad
TOK = HALO_ROWS * W  # 1904
QOFF = 3             # query rows sit at local rows 3..30

QH, QW = 14, 8       # query tile: 112 queries
NTI, NTJ = 2, 7      # tile grid per core
BR, BC = 20, 16      # padded key band (real cols: 14)
BAND = BR * BC       # 320
BR0 = {0: 0, 1: 14}                            # band local row start per ti
BC0 = [0, 5, 13, 21, 29, 37, 40]               # band local col start per tj
HCASE = [0, 1, 1, 1, 1, 1, 2]                  # tj -> horizontal case


def _win(p, n):
    """Clamped window start + bias row offset (matches reference _nbhd)."""
    start = np.clip(p - NS, 0, n - KS)
    ph = np.where(p < NS, KS - 1 - p, np.where(p + NS >= n, n - p - 1, NS))
    return start, ph


def _expb_tables(rpb, s):
    """exp(bias) * window-mask tables, [NTI, 3 hcases, NH, 112, BAND] f32.

    Key index within a band is k = r*BC + c (r: 20 band rows, c: 16 cols,
    last 2 cols are alignment padding -> always masked)."""
    out = np.zeros((NTI, 3, NH, 128, BAND), np.float32)
    qr = np.arange(QH)
    qc = np.arange(QW)
    r = np.arange(BR)
    c = np.arange(BC)
    for ti in range(NTI):
        g_qr = 28 * s + 14 * ti + qr                  # global query rows
        g_br = 28 * s - 3 + BR0[ti] + r               # global band rows
        vh_start, vh_ph = _win(g_qr, H)
        for hc, tj in ((0, 0), (1, 1), (2, 6)):
            g_qc = 8 * tj + qc
            g_bc = BC0[tj] + c
            vw_start, vw_ph = _win(g_qc, W)
            # valid[qr, r] / bias row index
            okr = (g_br[None, :] >= vh_start[:, None]) & (
                g_br[None, :] <= vh_start[:, None] + KS - 1)
            bi = vh_ph[:, None] + (g_br[None, :] - vh_start[:, None])
            okc = (g_bc[None, :] >= vw_start[:, None]) & (
                g_bc[None, :] <= vw_start[:, None] + KS - 1)
            bj = vw_ph[:, None] + (g_bc[None, :] - vw_start[:, None])
            bi = np.clip(bi, 0, 2 * KS - 2)
            bj = np.clip(bj, 0, 2 * KS - 2)
            for h in range(NH):
                bias = rpb[h][bi[:, None, :, None], bj[None, :, None, :]]
                val = np.exp(bias) * (okr[:, None, :, None] &
                                      okc[None, :, None, :])
                # [qr, qc, r, c] -> [q, k]
                out[ti, hc, h, :QH * QW] = val.reshape(QH * QW, BAND)
    return out


def _build_bass():
    import concourse.bass as bass
    import concourse.mybir as mybir
    import concourse.tile as tile
    from concourse import bacc

    fp32 = mybir.dt.float32
    bf16 = mybir.dt.bfloat16
    AF = mybir.ActivationFunctionType
    ALU = mybir.AluOpType

    nc = bacc.Bacc("TRN2", target_bir_lowering=False, debug=False)
    xT_d = nc.declare_dram_parameter("xT", [DIM, TOK], bf16, isOutput=False)
    wq_d = nc.declare_dram_parameter("wq", [DIM, DIM], bf16, isOutput=False)
    wk_d = nc.declare_dram_parameter("wk", [DIM, DIM], bf16, isOutput=False)
    wv_d = nc.declare_dram_parameter("wv", [DIM, DIM], bf16, isOutput=False)
    wp_d = nc.declare_dram_parameter("wp", [DIM, DIM], bf16, isOutput=False)
    bq_d = nc.declare_dram_parameter("bq", [DIM, 1], fp32, isOutput=False)
    eb_d = nc.declare_dram_parameter(
        "expB", [NTI * 3 * NH, 128, BAND], bf16, isOutput=False)
    out_d = nc.declare_dram_parameter(
        "out", [NTI * NTJ, QH * QW, DIM], fp32, isOutput=True)
    vnat_d = nc.dram_tensor("v_nat", [TOK + 4 * W, DIM], bf16)

    with tile.TileContext(nc) as tc:
        with tc.tile_pool(name="const", bufs=1) as cpool:
            wq = cpool.tile([DIM, DIM], bf16, tag="wq")
            wk = cpool.tile([DIM, DIM], bf16, tag="wk")
            wv = cpool.tile([DIM, DIM], bf16, tag="wv")
            wp = cpool.tile([DIM, DIM], bf16, tag="wp")
            bq = cpool.tile([DIM, 1], fp32, tag="bq")
            xT = cpool.tile([DIM, TOK], bf16, tag="xT")
            # qT is tile-major, 128-wide per tile (cols 112-127 zero pad
            # so score matmuls run M=128 with FWL weight loads)
            qT = cpool.tile([DIM, NTI * NTJ * 128], bf16, tag="qT")
            kT = cpool.tile([DIM, TOK], bf16, tag="kT")
            eb = cpool.tile([128, NTI * 3 * NH * BAND], bf16, tag="eb")
            for t, d in ((wq, wq_d), (wk, wk_d), (wv, wv_d), (wp, wp_d),
                         (bq, bq_d)):
                nc.sync.dma_start(t[:], d[:])
            for ci in range(4):
                sl = bass.ts(ci, TOK // 4)
                nc.sync.dma_start(xT[:, sl], xT_d[:, sl])
            # per-case loads so early tiles don't gate on the full table
            for cs in range(NTI * 3):
                nc.sync.dma_start(
                    eb[:, cs * NH * BAND:(cs + 1) * NH * BAND].rearrange(
                        "q (t k) -> q t k", t=NH),
                    eb_d[cs * NH:(cs + 1) * NH].rearrange(
                        "t q k -> q t k"))

            # ---- Stage A: qT / kT projections + v_nat to DRAM ----
            # bias goes through an ACT-local copy so consumers only need
            # same-engine FIFO ordering (instruction wait-slot limit).
            bq2 = cpool.tile([DIM, 1], fp32, tag="bq2")
            nc.scalar.copy(bq2[:], bq[:])
            nc.gpsimd.memset(
                qT[:].rearrange("p (t x) -> p t x", x=128)[:, :, QH * QW:],
                0.0)
            NCHUNK = 4
            CH = TOK // NCHUNK  # 476
            with (
                tc.tile_pool(name="psA", bufs=4, space="PSUM") as psA,
                tc.tile_pool(name="sbA", bufs=3) as sbA,
            ):
                for ci in range(NCHUNK):
                    sl = bass.ts(ci, CH)
                    pk = psA.tile([DIM, CH], fp32, tag="pq")
                    nc.tensor.matmul(pk[:], wk[:], xT[:, sl])
                    nc.scalar.copy(kT[:, sl], pk[:])
                for ci in range(4):  # q: half a tile-row (7 rows) per chunk
                    ti = ci // 2
                    r0 = QOFF + 14 * ti + 7 * (ci % 2)
                    pq = psA.tile([DIM, 7 * W], fp32, tag="pq")
                    nc.tensor.matmul(pq[:], wq[:],
                                     xT[:, r0 * W:r0 * W + 7 * W])
                    # escape reorders row-major (r, tj, qc) -> tile-major
                    qsl = qT[:, :].rearrange(
                        "p (t qr qc) -> p t qr qc", qr=16, qc=QW)[
                        :, NTJ * ti:NTJ * (ti + 1),
                        7 * (ci % 2):7 * (ci % 2) + 7, :]
                    pqv = pq[:, :].rearrange(
                        "p (r tj qc) -> p tj r qc", r=7, qc=QW)
                    nc.scalar.activation(qsl, pqv, AF.Identity,
                                         bias=bq2[:, 0:1])
                # v natural [tok, C]: PE -> PSUM -> DVE cast -> SBUF -> DRAM
                vsb = cpool.tile([128, 15 * DIM], bf16, tag="vsb")
                zpad = cpool.tile([128, 4 * W * DIM // 128], bf16,
                                  tag="zpad")
                nc.gpsimd.memset(zpad[:], 0.0)
                # tokens 1904-1919 of the last vsb chunk are junk: zero the
                # tail partitions (32-aligned start; real rows rewritten).
                nc.gpsimd.memset(vsb[96:, bass.ts(14, DIM)], 0.0)
                for ci in range(15):
                    t0 = ci * 128
                    tn = min(128, TOK - t0)
                    pv = psA.tile([128, DIM], fp32, tag="pv")
                    nc.tensor.matmul(pv[:tn, :], xT[:, t0:t0 + tn], wv[:])
                    nc.vector.tensor_copy(vsb[:tn, bass.ts(ci, DIM)],
                                          pv[:tn, :])
                nc.sync.dma_start(
                    vnat_d[:15 * 128, :].rearrange(
                        "(c p) d -> p c d", p=128),
                    vsb[:].rearrange("p (c d) -> p c d", d=DIM))
                nc.sync.dma_start(vnat_d[TOK:, :], zpad[:])


            # ---- Stage B: attention tiles ----
            with (
                tc.tile_pool(name="sb", bufs=10) as sb,
                tc.tile_pool(name="psS", bufs=3, space="PSUM") as psS,
                tc.tile_pool(name="psAV", bufs=1, space="PSUM") as psAV,
                tc.tile_pool(name="psP", bufs=1, space="PSUM") as psP,
            ):
                kT3 = kT[:].rearrange("p (r c) -> p r c", c=W)
                vnat3 = vnat_d[:].rearrange("(r c) d -> r c d", c=W)
                posb = cpool.tile([QH * QW, NTI * NTJ * DIM], fp32,
                                  tag="posb")

                # Software-pipelined by one tile: tile t's scores/exp/mask/
                # transpose (front) are emitted before tile t-1's AV/proj
                # (back), so the PE's in-order stream covers the ~1.3us
                # XBAR-transpose latency with the next tile's matmuls.
                def emit_front(t):
                    ti, tj = divmod(t, NTJ)
                    br0, bc0 = BR0[ti], BC0[tj]
                    case = (ti * 3 + HCASE[tj]) * NH
                    # v band gather; layout per 128-key chunk:
                    # [V_h0 |1| V_h1 |1| V_h2 |1| V_h3 |1] (33 cols each);
                    # ones columns ride the AV matmul as softmax sums.
                    vband = sb.tile([128, 3 * 132], bf16, tag="vband")
                    nc.gpsimd.memset(
                        vband[:].rearrange(
                            "p (x u) -> p x u", u=33)[:, :, HD:], 1.0)
                    for j, rn in ((0, 8), (1, 8), (2, 4)):
                        nc.sync.dma_start(
                            vband[:rn * BC, :].rearrange(
                                "p (c h d) -> p c h d", c=3, h=NH)[
                                :, j, :, :HD],
                            vnat3[br0 + 8 * j:br0 + 8 * j + rn,
                                  bc0:bc0 + BC, :].rearrange(
                                "r c (h d) -> r c h d", h=NH))
                    P = sb.tile([128, 4 * 3 * DIM], bf16, tag="P")
                    nc.gpsimd.memset(
                        P[:].rearrange("p (x u) -> p x u", u=3 * DIM)[
                            :, :, BAND:], 0.0)
                    for hp in range(2):
                        S = psS.tile([128, 1024], fp32, tag="S")
                        for i in range(2):
                            h = 2 * hp + i
                            nc.tensor.matmul(
                                S[:, 512 * i:512 * i + BAND],
                                qT[32 * h:32 * h + 32,
                                   bass.ts(t, 128)],
                                kT3[32 * h:32 * h + 32,
                                    br0:br0 + BR, bc0:bc0 + BC],
                                tile_position=(32 * h, 0))
                        expS = sb.tile([128, 2 * BAND], bf16,
                                       tag="expS")
                        nc.scalar.activation(
                            expS[:].rearrange("q (h k) -> q h k", h=2),
                            S[:].rearrange(
                                "q (h k) -> q h k", h=2)[:, :, :BAND],
                            AF.Exp)
                        for i in range(2):
                            h = 2 * hp + i
                            nc.vector.tensor_tensor(
                                P[:, 384 * h:384 * h + BAND],
                                expS[:, bass.ts(i, BAND)],
                                eb[:, (case + h) * BAND:
                                   (case + h + 1) * BAND],
                                ALU.mult)
                    PT = sb.tile([128, 4 * 3 * DIM], bf16, tag="PT")
                    nc.sync.dma_start(
                        PT[:].rearrange("p (j q) -> p j q", j=12),
                        P[:], transpose=True)
                    return vband, PT

                def emit_proj(t, attnT):
                    po = psP.tile([128, DIM], fp32, tag="po")
                    nc.tensor.matmul(po[:], attnT[:], wp[:])
                    nc.scalar.copy(posb[:, bass.ts(t, DIM)],
                                   po[:QH * QW, :])

                def emit_back(t, vband, PT):
                    attn = sb.tile([QH * QW, DIM + 16], bf16, tag="attn")
                    for hp in range(2):
                        rs = sb.tile([QH * QW, 2], fp32, tag="rs")
                        av = psAV.tile([128, 2 * 33], fp32, tag="av")
                        for i in range(2):
                            h = 2 * hp + i
                            for j, kn in ((0, 128), (1, 128), (2, 64)):
                                nc.tensor.matmul(
                                    av[:, bass.ts(i, 33)],
                                    PT[:kn, bass.ts(3 * h + j, 128)],
                                    vband[:kn, 132 * j + 33 * h:
                                          132 * j + 33 * (h + 1)],
                                    start=(j == 0), stop=(j == 2))
                        # col 32 of each av block = sum(exp) per query
                        nc.vector.reciprocal(
                            rs[:].rearrange("q (i u) -> q i u", u=1),
                            av[:QH * QW, :].rearrange(
                                "q (i d) -> q i d", i=2)[:, :, HD:HD + 1])
                        for i in range(2):
                            h = 2 * hp + i
                            nc.vector.tensor_scalar_mul(
                                attn[:, HD * h:HD * (h + 1)],
                                av[:QH * QW, 33 * i:33 * i + HD],
                                rs[:, i:i + 1])
                    attnT = sb.tile([128, 128], bf16, tag="attnT")
                    nc.vector.memset(attnT[:, QH * QW:], 0.0)
                    nc.sync.dma_start(attnT[:, :QH * QW], attn[:, :DIM],
                                      transpose=True)
                    return attnT
                NT = NTI * NTJ
                fronts, backs = {}, {}
                for t in range(NT + 2):
                    if t < NT:
                        fronts[t] = emit_front(t)
                    if 1 <= t <= NT:
                        backs[t - 1] = emit_back(t - 1, *fronts.pop(t - 1))
                    if t >= 2:
                        emit_proj(t - 2, backs.pop(t - 2))
                nc.sync.dma_start(
                    out_d[:].rearrange("t q d -> q t d"),
                    posb[:].rearrange("q (t d) -> q t d", d=DIM))
    nc.compile()
    return nc


_CACHE = {}


def _get_runner():
    """Compile the SPMD program once; returns (run_fn, bench_fn)."""
    if "runner" in _CACHE:
        return _CACHE["runner"]
    import jax
    from jax.experimental.shard_map import shard_map
    from jax.sharding import Mesh, NamedSharding, PartitionSpec
    import concourse.mybir as mybir
    from concourse import bass2jax

    bass2jax.install_neuronx_cc_hook()
    nc = _build_bass()
    n_cores = 8

    in_names, out_names, out_avals = [], [], []
    for alloc in nc.m.functions[0].allocations:
        if not isinstance(alloc, mybir.MemoryLocationSet):
            continue
        name = alloc.memorylocations[0].name
        if alloc.kind == "ExternalInput":
            in_names.append(name)
        elif alloc.kind == "ExternalOutput":
            out_names.append(name)
            out_avals.append(jax.core.ShapedArray(
                tuple(alloc.tensor_shape), mybir.dt.np(alloc.dtype)))
    part_name = (nc.partition_id_tensor.name
                 if nc.partition_id_tensor else None)
    if part_name in in_names:
        in_names.remove(part_name)
    n_params = len(in_names)
    all_names = in_names + out_names
    if part_name is not None:
        all_names = all_names + [part_name]

    def _body(*args):
        operands = list(args)
        if part_name is not None:
            operands.append(bass2jax.partition_id_tensor())
        return tuple(bass2jax._bass_exec_p.bind(
            *operands, out_avals=tuple(out_avals), in_names=tuple(all_names),
            out_names=tuple(out_names), lowering_input_output_aliases=(),
            sim_require_finite=True, sim_require_nnan=True, nc=nc))

    devices = jax.devices()[:n_cores]
    mesh = Mesh(np.asarray(devices), ("core",))
    spec = PartitionSpec("core")
    n_args = n_params + len(out_names)
    sharded = jax.jit(
        shard_map(_body, mesh=mesh, in_specs=(spec,) * n_args,
                  out_specs=(spec,) * len(out_names), check_rep=False),
        keep_unused=True)
    sharding = NamedSharding(mesh, spec)

    def _put(in_maps):
        arrs = []
        for i, name in enumerate(in_names):
            arrs.append(jax.device_put(np.concatenate(
                [in_maps[c][name] for c in range(n_cores)], axis=0),
                sharding))
        for av in out_avals:
            arrs.append(jax.device_put(
                np.zeros((n_cores * av.shape[0], *av.shape[1:]), av.dtype),
                sharding))
        return arrs

    def run_fn(in_maps):
        arrs = _put(in_maps)
        outs = sharded(*arrs)
        return [
            {name: np.asarray(outs[i]).reshape(n_cores, *out_avals[i].shape)[c]
             for i, name in enumerate(out_names)}
            for c in range(n_cores)
        ]

    def bench_fn(in_maps, iters=6, burst=8):
        # Amortized per-execution time: queue `burst` async dispatches and
        # block once; slope vs a single dispatch removes the host/axon
        # round-trip constant.
        import time
        arrs = _put(in_maps)
        jax.block_until_ready(sharded(*arrs))
        b1 = bn = float("inf")
        for _ in range(iters):
            t0 = time.perf_counter()
            jax.block_until_ready(sharded(*arrs))
            b1 = min(b1, time.perf_counter() - t0)
            t0 = time.perf_counter()
            outs = [sharded(*arrs) for _ in range(burst)]
            jax.block_until_ready(outs)
            bn = min(bn, time.perf_counter() - t0)
        per = (bn - b1) / (burst - 1)
        return max(per, 0.0) * 1e9
    _CACHE["runner"] = (run_fn, bench_fn)
    return _CACHE["runner"]


def _make_in_maps(x, w_qkv, b_qkv, rpb, w_proj, b_proj):
    x = np.asarray(x, np.float32)
    w_qkv = np.asarray(w_qkv, np.float32)
    b_qkv = np.asarray(b_qkv, np.float32)
    rpb = np.asarray(rpb, np.float32)
    w_proj = np.asarray(w_proj, np.float32)
    b_proj = np.asarray(b_proj, np.float32)

    wq = (w_qkv[:, 0:128] * SCALE).astype(ml_dtypes.bfloat16)
    wk = w_qkv[:, 128:256].astype(ml_dtypes.bfloat16)
    wv = w_qkv[:, 256:384].astype(ml_dtypes.bfloat16)
    wp = w_proj.astype(ml_dtypes.bfloat16)
    bq = (b_qkv[0:128] * SCALE).astype(np.float32).reshape(128, 1)
    # k-bias: softmax-invariant, dropped. v-bias folds into output bias.
    out_bias = b_proj + b_qkv[256:384] @ w_proj

    in_maps = []
    for core in range(8):
        b, s = divmod(core, 2)
        lo = 28 * s - 3
        rows = np.clip(np.arange(lo, lo + HALO_ROWS), 0, H - 1)
        xs = x[b, rows, :, :]                       # [34, 56, 128]
        if s == 0:
            xs[0:3] = 0.0
        else:
            xs[31:34] = 0.0
        xT = np.ascontiguousarray(
            xs.reshape(TOK, DIM).T).astype(ml_dtypes.bfloat16)
        expB = _expb_tables(rpb, s).reshape(
            NTI * 3 * NH, 128, BAND).astype(ml_dtypes.bfloat16)
        in_maps.append(dict(xT=xT, wq=wq, wk=wk, wv=wv, wp=wp, bq=bq,
                            expB=expB))
    return in_maps, out_bias


def _gather(results, out_bias):
    out = np.zeros((B, H, W, DIM), np.float32)
    for core in range(8):
        b, s = divmod(core, 2)
        o = results[core]["out"].reshape(NTI, NTJ, QH, QW, DIM)
        o = o.transpose(0, 2, 1, 3, 4).reshape(SLAB, W, DIM)
        out[b, 28 * s:28 * (s + 1)] = o
    out += out_bias
    return out


def kernel(x, w_qkv, b_qkv, rpb, w_proj, b_proj):
    in_maps, out_bias = _make_in_maps(x, w_qkv, b_qkv, rpb, w_proj, b_proj)
    run_fn, _ = _get_runner()
    return _gather(run_fn(in_maps), out_bias)


def bench(x, w_qkv, b_qkv, rpb, w_proj, b_proj, iters=20):
    """Returns (output, best_wall_ns) using the cached compiled runner."""
    in_maps, out_bias = _make_in_maps(x, w_qkv, b_qkv, rpb, w_proj, b_proj)
    run_fn, bench_fn = _get_runner()
    out = _gather(run_fn(in_maps), out_bias)
    return out, bench_fn(in_maps, iters)


# revision 57
# speedup vs baseline: 1.1441x; 1.1441x over previous
"""Neighborhood attention (NATTEN-style 7x7 window, 4 heads, rpb) on 8
Trainium2 NeuronCores via Bass/Tile.

Sharding: 8 cores = 4 batches x 2 row-slabs (rows 0-27 / 28-55). Each core
gets a 34-row halo'd slab (3 zero pad rows at one edge so both slab variants
share one SPMD program), host-transposed to xT [C=128, 1904 tokens] bf16.

Per-core dataflow:
  qT = (x Wq*scale)^T  tile-major     (PE per 14x8 query tile; heads stacked
                                       on partitions, ACT escape w/ q-bias)
  kT = (x Wk)^T  row-major            (PE + ACT cast escape)
  v_nat = x Wv  [token, C] -> DRAM    (PE + DVE cast, one staging store)
  per 14x8 query tile (112 queries, padded 20x16 key band = 320 keys):
    scores[112,320] x4 heads          (PE, one matmul per head packed into
                                       32-row groups via tile_position ->
                                       4 PSUM banks)
    expS = exp(scores)                (ACT, 2 heads per op, PSUM->SBUF bf16)
    P = expS * expB                   (DVE; expB = host table exp(bias) *
                                       window mask, 0 outside the window ->
                                       masking and bias in one multiply)
    P^T                               (one chunked XBAR DMA-transpose
                                       [128,384] -> 3x[128,128])
    av = P^T.T @ [V|1] per 128-chunk  (PE; v_band gathered from v_nat DRAM
                                       with an interleaved ones column ->
                                       col 32 = softmax denominator)
    attn = av[:, :32] * recip(av[:,32])  (DVE)
  attnT = DMA-transpose(attn [112,128])
  out = attnT.T @ Wp -> f32 -> HBM    (PE + ACT escape, single store)

b_qkv: k-bias is softmax-invariant (dropped); v-bias folds into the output
bias host-side (sum(attn)=1); q-bias applied in the qT escape. Softmax skips
the max-subtraction (|logits| <~ 30 here, exp is safe in f32->bf16).
"""

import numpy as np
import ml_dtypes

KS = 7
NS = KS // 2
NH = 4
DIM = 128
HD = DIM // NH
SCALE = HD ** -0.5

B, H, W = 4, 56, 56
SLAB = 28            # query rows per core
HALO_ROWS = 34       # 28 + 3 halo + 3 uniform pad
TOK = HALO_ROWS * W  # 1904
QOFF = 3             # query rows sit at local rows 3..30

QH, QW = 14, 8       # query tile: 112 queries
NTI, NTJ = 2, 7      # tile grid per core
BR, BC = 20, 16      # padded key band (real cols: 14)
BAND = BR * BC       # 320
BR0 = {0: 0, 1: 14}                            # band local row start per ti
BC0 = [0, 5, 13, 21, 29, 37, 40]               # band local col start per tj
HCASE = [0, 1, 1, 1, 1, 1, 2]                  # tj -> horizontal case


def _win(p, n):
    """Clamped window start + bias row offset (matches reference _nbhd)."""
    start = np.clip(p - NS, 0, n - KS)
    ph = np.where(p < NS, KS - 1 - p, np.where(p + NS >= n, n - p - 1, NS))
    return start, ph


def _expb_tables(rpb, s):
    """exp(bias) * window-mask tables, [NTI, 3 hcases, NH, 112, BAND] f32.

    Key index within a band is k = r*BC + c (r: 20 band rows, c: 16 cols,
    last 2 cols are alignment padding -> always masked)."""
    out = np.zeros((NTI, 3, NH, 128, BAND), np.float32)
    qr = np.arange(QH)
    qc = np.arange(QW)
    r = np.arange(BR)
    c = np.arange(BC)
    for ti in range(NTI):
        g_qr = 28 * s + 14 * ti + qr                  # global query rows
        g_br = 28 * s - 3 + BR0[ti] + r               # global band rows
        vh_start, vh_ph = _win(g_qr, H)
        for hc, tj in ((0, 0), (1, 1), (2, 6)):
            g_qc = 8 * tj + qc
            g_bc = BC0[tj] + c
            vw_start, vw_ph = _win(g_qc, W)
            # valid[qr, r] / bias row index
            okr = (g_br[None, :] >= vh_start[:, None]) & (
                g_br[None, :] <= vh_start[:, None] + KS - 1)
            bi = vh_ph[:, None] + (g_br[None, :] - vh_start[:, None])
            okc = (g_bc[None, :] >= vw_start[:, None]) & (
                g_bc[None, :] <= vw_start[:, None] + KS - 1)
            bj = vw_ph[:, None] + (g_bc[None, :] - vw_start[:, None])
            bi = np.clip(bi, 0, 2 * KS - 2)
            bj = np.clip(bj, 0, 2 * KS - 2)
            for h in range(NH):
                bias = rpb[h][bi[:, None, :, None], bj[None, :, None, :]]
                val = np.exp(bias) * (okr[:, None, :, None] &
                                      okc[None, :, None, :])
                # [qr, qc, r, c] -> [q, k]
                out[ti, hc, h, :QH * QW] = val.reshape(QH * QW, BAND)
    return out


def _build_bass():
    import concourse.bass as bass
    import concourse.mybir as mybir
    import concourse.tile as tile
    from concourse import bacc

    fp32 = mybir.dt.float32
    bf16 = mybir.dt.bfloat16
    AF = mybir.ActivationFunctionType
    ALU = mybir.AluOpType

    nc = bacc.Bacc("TRN2", target_bir_lowering=False, debug=False)
    xT_d = nc.declare_dram_parameter("xT", [DIM, TOK], bf16, isOutput=False)
    wq_d = nc.declare_dram_parameter("wq", [DIM, DIM], bf16, isOutput=False)
    wk_d = nc.declare_dram_parameter("wk", [DIM, DIM], bf16, isOutput=False)
    wv_d = nc.declare_dram_parameter("wv", [DIM, DIM], bf16, isOutput=False)
    wp_d = nc.declare_dram_parameter("wp", [DIM, DIM], bf16, isOutput=False)
    bq_d = nc.declare_dram_parameter("bq", [DIM, 1], fp32, isOutput=False)
    eb_d = nc.declare_dram_parameter(
        "expB", [NTI * 3 * NH, 128, BAND], bf16, isOutput=False)
    out_d = nc.declare_dram_parameter(
        "out", [NTI * NTJ, QH * QW, DIM], fp32, isOutput=True)
    vnat_d = nc.dram_tensor("v_nat", [TOK + 4 * W, DIM], bf16)

    with tile.TileContext(nc) as tc:
        with tc.tile_pool(name="const", bufs=1) as cpool:
            wq = cpool.tile([DIM, DIM], bf16, tag="wq")
            wk = cpool.tile([DIM, DIM], bf16, tag="wk")
            wv = cpool.tile([DIM, DIM], bf16, tag="wv")
            wp = cpool.tile([DIM, DIM], bf16, tag="wp")
            bq = cpool.tile([DIM, 1], fp32, tag="bq")
            xT = cpool.tile([DIM, TOK], bf16, tag="xT")
            # qT is tile-major, 128-wide per tile (cols 112-127 zero pad
            # so score matmuls run M=128 with FWL weight loads)
            qT = cpool.tile([DIM, NTI * NTJ * 128], bf16, tag="qT")
            kT = cpool.tile([DIM, TOK], bf16, tag="kT")
            eb = cpool.tile([128, NTI * 3 * NH * BAND], bf16, tag="eb")
            for t, d in ((wq, wq_d), (wk, wk_d), (wv, wv_d), (wp, wp_d),
                         (bq, bq_d)):
                nc.sync.dma_start(t[:], d[:])
            for ci in range(4):
                sl = bass.ts(ci, TOK // 4)
                nc.sync.dma_start(xT[:, sl], xT_d[:, sl])
            # per-case loads so early tiles don't gate on the full table
            for cs in range(NTI * 3):
                nc.sync.dma_start(
                    eb[:, cs * NH * BAND:(cs + 1) * NH * BAND].rearrange(
                        "q (t k) -> q t k", t=NH),
                    eb_d[cs * NH:(cs + 1) * NH].rearrange(
                        "t q k -> q t k"))

            # ---- Stage A: qT / kT projections + v_nat to DRAM ----
            # bias goes through an ACT-local copy so consumers only need
            # same-engine FIFO ordering (instruction wait-slot limit).
            bq2 = cpool.tile([DIM, 1], fp32, tag="bq2")
            nc.scalar.copy(bq2[:], bq[:])
            nc.gpsimd.memset(
                qT[:].rearrange("p (t x) -> p t x", x=128)[:, :, QH * QW:],
                0.0)
            NCHUNK = 4
            CH = TOK // NCHUNK  # 476
            with (
                tc.tile_pool(name="psA", bufs=4, space="PSUM") as psA,
                tc.tile_pool(name="sbA", bufs=3) as sbA,
            ):
                for ci in range(NCHUNK):
                    sl = bass.ts(ci, CH)
                    pk = psA.tile([DIM, CH], fp32, tag="pq")
                    nc.tensor.matmul(pk[:], wk[:], xT[:, sl])
                    nc.scalar.copy(kT[:, sl], pk[:])
                for ci in range(4):  # q: half a tile-row (7 rows) per chunk
                    ti = ci // 2
                    r0 = QOFF + 14 * ti + 7 * (ci % 2)
                    pq = psA.tile([DIM, 7 * W], fp32, tag="pq")
                    nc.tensor.matmul(pq[:], wq[:],
                                     xT[:, r0 * W:r0 * W + 7 * W])
                    # escape reorders row-major (r, tj, qc) -> tile-major
                    qsl = qT[:, :].rearrange(
                        "p (t qr qc) -> p t qr qc", qr=16, qc=QW)[
                        :, NTJ * ti:NTJ * (ti + 1),
                        7 * (ci % 2):7 * (ci % 2) + 7, :]
                    pqv = pq[:, :].rearrange(
                        "p (r tj qc) -> p tj r qc", r=7, qc=QW)
                    nc.scalar.activation(qsl, pqv, AF.Identity,
                                         bias=bq2[:, 0:1])
                # v natural [tok, C]: PE -> PSUM -> DVE cast -> SBUF -> DRAM
                vsb = cpool.tile([128, 15 * DIM], bf16, tag="vsb")
                zpad = cpool.tile([128, 4 * W * DIM // 128], bf16,
                                  tag="zpad")
                nc.gpsimd.memset(zpad[:], 0.0)
                # tokens 1904-1919 of the last vsb chunk are junk: zero the
                # tail partitions (32-aligned start; real rows rewritten).
                nc.gpsimd.memset(vsb[96:, bass.ts(14, DIM)], 0.0)
                for ci in range(15):
                    t0 = ci * 128
                    tn = min(128, TOK - t0)
                    pv = psA.tile([128, DIM], fp32, tag="pv")
                    nc.tensor.matmul(pv[:tn, :], xT[:, t0:t0 + tn], wv[:])
                    nc.vector.tensor_copy(vsb[:tn, bass.ts(ci, DIM)],
                                          pv[:tn, :])
                nc.sync.dma_start(
                    vnat_d[:15 * 128, :].rearrange(
                        "(c p) d -> p c d", p=128),
                    vsb[:].rearrange("p (c d) -> p c d", d=DIM))
                nc.sync.dma_start(vnat_d[TOK:, :], zpad[:])


            # ---- Stage B: attention tiles ----
            with (
                tc.tile_pool(name="sb", bufs=10) as sb,
                tc.tile_pool(name="psS", bufs=3, space="PSUM") as psS,
                tc.tile_pool(name="psAV", bufs=1, space="PSUM") as psAV,
                tc.tile_pool(name="psP", bufs=1, space="PSUM") as psP,
            ):
                kT3 = kT[:].rearrange("p (r c) -> p r c", c=W)
                vnat3 = vnat_d[:].rearrange("(r c) d -> r c d", c=W)
                posb = cpool.tile([QH * QW, NTI * NTJ * DIM], fp32,
                                  tag="posb")

                # Software-pipelined by one tile: tile t's scores/exp/mask/
                # transpose (front) are emitted before tile t-1's AV/proj
                # (back), so the PE's in-order stream covers the ~1.3us
                # XBAR-transpose latency with the next tile's matmuls.
                def emit_front(t):
                    ti, tj = divmod(t, NTJ)
                    br0, bc0 = BR0[ti], BC0[tj]
                    case = (ti * 3 + HCASE[tj]) * NH
                    # v band gather; layout per 128-key chunk:
                    # [V_h0 |1| V_h1 |1| V_h2 |1| V_h3 |1] (33 cols each);
                    # ones columns ride the AV matmul as softmax sums.
                    vband = sb.tile([128, 3 * 132], bf16, tag="vband")
                    nc.gpsimd.memset(
                        vband[:].rearrange(
                            "p (x u) -> p x u", u=33)[:, :, HD:], 1.0)
                    for j, rn in ((0, 8), (1, 8), (2, 4)):
                        nc.sync.dma_start(
                            vband[:rn * BC, :].rearrange(
                                "p (c h d) -> p c h d", c=3, h=NH)[
                                :, j, :, :HD],
                            vnat3[br0 + 8 * j:br0 + 8 * j + rn,
                                  bc0:bc0 + BC, :].rearrange(
                                "r c (h d) -> r c h d", h=NH))
                    P = sb.tile([128, 4 * 3 * DIM], bf16, tag="P")
                    nc.gpsimd.memset(
                        P[:].rearrange("p (x u) -> p x u", u=3 * DIM)[
                            :, :, BAND:], 0.0)
                    for hp in range(2):
                        S = psS.tile([128, 1024], fp32, tag="S")
                        for i in range(2):
                            h = 2 * hp + i
                            nc.tensor.matmul(
                                S[:, 512 * i:512 * i + BAND],
                                qT[32 * h:32 * h + 32,
                                   bass.ts(t, 128)],
                                kT3[32 * h:32 * h + 32,
                                    br0:br0 + BR, bc0:bc0 + BC],
                                tile_position=(32 * h, 0))
                        expS = sb.tile([128, 2 * BAND], bf16,
                                       tag="expS")
                        nc.scalar.activation(
                            expS[:].rearrange("q (h k) -> q h k", h=2),
                            S[:].rearrange(
                                "q (h k) -> q h k", h=2)[:, :, :BAND],
                            AF.Exp)
                        for i in range(2):
                            h = 2 * hp + i
                            nc.vector.tensor_tensor(
                                P[:, 384 * h:384 * h + BAND],
                                expS[:, bass.ts(i, BAND)],
                                eb[:, (case + h) * BAND:
                                   (case + h + 1) * BAND],
                                ALU.mult)
                    PT = sb.tile([128, 4 * 3 * DIM], bf16, tag="PT")
                    nc.sync.dma_start(
                        PT[:].rearrange("p (j q) -> p j q", j=12),
                        P[:], transpose=True)
                    return vband, PT

                def emit_proj(t, attnT):
                    po = psP.tile([128, DIM], fp32, tag="po")
                    nc.tensor.matmul(po[:], attnT[:], wp[:])
                    nc.scalar.copy(posb[:, bass.ts(t, DIM)],
                                   po[:QH * QW, :])

                def emit_back(t, vband, PT):
                    attn = sb.tile([QH * QW, DIM + 16], bf16, tag="attn")
                    for hp in range(2):
                        rs = sb.tile([QH * QW, 2], fp32, tag="rs")
                        av = psAV.tile([128, 2 * 33], fp32, tag="av")
                        for i in range(2):
                            h = 2 * hp + i
                            for j, kn in ((0, 128), (1, 128), (2, 64)):
                                nc.tensor.matmul(
                                    av[:, bass.ts(i, 33)],
                                    PT[:kn, bass.ts(3 * h + j, 128)],
                                    vband[:kn, 132 * j + 33 * h:
                                          132 * j + 33 * (h + 1)],
                                    start=(j == 0), stop=(j == 2))
                        # col 32 of each av block = sum(exp) per query
                        nc.vector.reciprocal(
                            rs[:].rearrange("q (i u) -> q i u", u=1),
                            av[:QH * QW, :].rearrange(
                                "q (i d) -> q i d", i=2)[:, :, HD:HD + 1])
                        for i in range(2):
                            h = 2 * hp + i
                            nc.vector.tensor_scalar_mul(
                                attn[:, HD * h:HD * (h + 1)],
                                av[:QH * QW, 33 * i:33 * i + HD],
                                rs[:, i:i + 1])
                    attnT = sb.tile([128, 128], bf16, tag="attnT")
                    nc.vector.memset(attnT[:, QH * QW:], 0.0)
                    nc.sync.dma_start(attnT[:, :QH * QW], attn[:, :DIM],
                                      transpose=True)
                    return attnT
                NT = NTI * NTJ
                fronts, backs = {}, {}
                for t in range(NT + 2):
                    if t < NT:
                        fronts[t] = emit_front(t)
                    if 1 <= t <= NT:
                        backs[t - 1] = emit_back(t - 1, *fronts.pop(t - 1))
                    if t >= 2:
                        emit_proj(t - 2, backs.pop(t - 2))
                nc.sync.dma_start(
                    out_d[:].rearrange("t q d -> q t d"),
                    posb[:].rearrange("q (t d) -> q t d", d=DIM))
    nc.compile()
    return nc


_CACHE = {}


def _get_runner():
    """Compile the SPMD program once; returns (run_fn, bench_fn)."""
    if "runner" in _CACHE:
        return _CACHE["runner"]
    import jax
    from jax.experimental.shard_map import shard_map
    from jax.sharding import Mesh, NamedSharding, PartitionSpec
    import concourse.mybir as mybir
    from concourse import bass2jax

    bass2jax.install_neuronx_cc_hook()
    nc = _build_bass()
    n_cores = 8

    in_names, out_names, out_avals = [], [], []
    for alloc in nc.m.functions[0].allocations:
        if not isinstance(alloc, mybir.MemoryLocationSet):
            continue
        name = alloc.memorylocations[0].name
        if alloc.kind == "ExternalInput":
            in_names.append(name)
        elif alloc.kind == "ExternalOutput":
            out_names.append(name)
            out_avals.append(jax.core.ShapedArray(
                tuple(alloc.tensor_shape), mybir.dt.np(alloc.dtype)))
    part_name = (nc.partition_id_tensor.name
                 if nc.partition_id_tensor else None)
    if part_name in in_names:
        in_names.remove(part_name)
    n_params = len(in_names)
    all_names = in_names + out_names
    if part_name is not None:
        all_names = all_names + [part_name]

    def _body(*args):
        operands = list(args)
        if part_name is not None:
            operands.append(bass2jax.partition_id_tensor())
        return tuple(bass2jax._bass_exec_p.bind(
            *operands, out_avals=tuple(out_avals), in_names=tuple(all_names),
            out_names=tuple(out_names), lowering_input_output_aliases=(),
            sim_require_finite=True, sim_require_nnan=True, nc=nc))

    devices = jax.devices()[:n_cores]
    mesh = Mesh(np.asarray(devices), ("core",))
    spec = PartitionSpec("core")
    n_args = n_params + len(out_names)
    sharded = jax.jit(
        shard_map(_body, mesh=mesh, in_specs=(spec,) * n_args,
                  out_specs=(spec,) * len(out_names), check_rep=False),
        keep_unused=True)
    sharding = NamedSharding(mesh, spec)

    def _put(in_maps):
        arrs = []
        for i, name in enumerate(in_names):
            arrs.append(jax.device_put(np.concatenate(
                [in_maps[c][name] for c in range(n_cores)], axis=0),
                sharding))
        for av in out_avals:
            arrs.append(jax.device_put(
                np.zeros((n_cores * av.shape[0], *av.shape[1:]), av.dtype),
                sharding))
        return arrs

    def run_fn(in_maps):
        arrs = _put(in_maps)
        outs = sharded(*arrs)
        return [
            {name: np.asarray(outs[i]).reshape(n_cores, *out_avals[i].shape)[c]
             for i, name in enumerate(out_names)}
            for c in range(n_cores)
        ]

    def bench_fn(in_maps, iters=6, burst=8):
        # Amortized per-execution time: queue `burst` async dispatches and
        # block once; slope vs a single dispatch removes the host/axon
        # round-trip constant.
        import time
        arrs = _put(in_maps)
        jax.block_until_ready(sharded(*arrs))
        b1 = bn = float("inf")
        for _ in range(iters):
            t0 = time.perf_counter()
            jax.block_until_ready(sharded(*arrs))
            b1 = min(b1, time.perf_counter() - t0)
            t0 = time.perf_counter()
            outs = [sharded(*arrs) for _ in range(burst)]
            jax.block_until_ready(outs)
            bn = min(bn, time.perf_counter() - t0)
        per = (bn - b1) / (burst - 1)
        return max(per, 0.0) * 1e9
    _CACHE["runner"] = (run_fn, bench_fn)
    return _CACHE["runner"]


def _make_in_maps(x, w_qkv, b_qkv, rpb, w_proj, b_proj):
    x = np.asarray(x, np.float32)
    w_qkv = np.asarray(w_qkv, np.float32)
    b_qkv = np.asarray(b_qkv, np.float32)
    rpb = np.asarray(rpb, np.float32)
    w_proj = np.asarray(w_proj, np.float32)
    b_proj = np.asarray(b_proj, np.float32)

    wq = (w_qkv[:, 0:128] * SCALE).astype(ml_dtypes.bfloat16)
    wk = w_qkv[:, 128:256].astype(ml_dtypes.bfloat16)
    wv = w_qkv[:, 256:384].astype(ml_dtypes.bfloat16)
    wp = w_proj.astype(ml_dtypes.bfloat16)
    bq = (b_qkv[0:128] * SCALE).astype(np.float32).reshape(128, 1)
    # k-bias: softmax-invariant, dropped. v-bias folds into output bias.
    out_bias = b_proj + b_qkv[256:384] @ w_proj

    in_maps = []
    for core in range(8):
        b, s = divmod(core, 2)
        lo = 28 * s - 3
        rows = np.clip(np.arange(lo, lo + HALO_ROWS), 0, H - 1)
        xs = x[b, rows, :, :]                       # [34, 56, 128]
        if s == 0:
            xs[0:3] = 0.0
        else:
            xs[31:34] = 0.0
        xT = np.ascontiguousarray(
            xs.reshape(TOK, DIM).T).astype(ml_dtypes.bfloat16)
        expB = _expb_tables(rpb, s).reshape(
            NTI * 3 * NH, 128, BAND).astype(ml_dtypes.bfloat16)
        in_maps.append(dict(xT=xT, wq=wq, wk=wk, wv=wv, wp=wp, bq=bq,
                            expB=expB))
    return in_maps, out_bias


def _gather(results, out_bias):
    out = np.zeros((B, H, W, DIM), np.float32)
    for core in range(8):
        b, s = divmod(core, 2)
        o = results[core]["out"].reshape(NTI, NTJ, QH, QW, DIM)
        o = o.transpose(0, 2, 1, 3, 4).reshape(SLAB, W, DIM)
        out[b, 28 * s:28 * (s + 1)] = o
    out += out_bias
    return out


def kernel(x, w_qkv, b_qkv, rpb, w_proj, b_proj):
    in_maps, out_bias = _make_in_maps(x, w_qkv, b_qkv, rpb, w_proj, b_proj)
    run_fn, _ = _get_runner()
    return _gather(run_fn(in_maps), out_bias)


def bench(x, w_qkv, b_qkv, rpb, w_proj, b_proj, iters=20):
    """Returns (output, best_wall_ns) using the cached compiled runner."""
    in_maps, out_bias = _make_in_maps(x, w_qkv, b_qkv, rpb, w_proj, b_proj)
    run_fn, bench_fn = _get_runner()
    out = _gather(run_fn(in_maps), out_bias)
    return out, bench_fn(in_maps, iters)


# revision 58
# speedup vs baseline: 1.2730x; 1.1127x over previous
"""Neighborhood attention (NATTEN-style 7x7 window, 4 heads, rpb) on 8
Trainium2 NeuronCores via Bass/Tile.

Sharding: 8 cores = 4 batches x 2 row-slabs (rows 0-27 / 28-55). Each core
gets a 34-row halo'd slab (3 zero pad rows at one edge so both slab variants
share one SPMD program), host-transposed to xT [C=128, 1904 tokens] bf16.

Per-core dataflow:
  qT = (x Wq*scale)^T  tile-major     (PE per 14x8 query tile; heads stacked
                                       on partitions, ACT escape w/ q-bias)
  kT = (x Wk)^T  row-major            (PE + ACT cast escape)
  v_nat = x Wv  [token, C] -> DRAM    (PE + DVE cast, one staging store)
  per 14x8 query tile (112 queries, padded 20x16 key band = 320 keys):
    scores[112,320] x4 heads          (PE, one matmul per head packed into
                                       32-row groups via tile_position ->
                                       4 PSUM banks)
    expS = exp(scores)                (ACT, 2 heads per op, PSUM->SBUF bf16)
    P = expS * expB                   (DVE; expB = host table exp(bias) *
                                       window mask, 0 outside the window ->
                                       masking and bias in one multiply)
    P^T                               (one chunked XBAR DMA-transpose
                                       [128,384] -> 3x[128,128])
    av = P^T.T @ [V|1] per 128-chunk  (PE; v_band gathered from v_nat DRAM
                                       with an interleaved ones column ->
                                       col 32 = softmax denominator)
    attn = av[:, :32] * recip(av[:,32])  (DVE)
  attnT = DMA-transpose(attn [112,128])
  out = attnT.T @ Wp -> f32 -> HBM    (PE + ACT escape, single store)

b_qkv: k-bias is softmax-invariant (dropped); v-bias folds into the output
bias host-side (sum(attn)=1); q-bias applied in the qT escape. Softmax skips
the max-subtraction (|logits| <~ 30 here, exp is safe in f32->bf16).
"""

import numpy as np
import ml_dtypes

KS = 7
NS = KS // 2
NH = 4
DIM = 128
HD = DIM // NH
SCALE = HD ** -0.5

B, H, W = 4, 56, 56
SLAB = 28            # query rows per core
HALO_ROWS = 34       # 28 + 3 halo + 3 uniform pad
TOK = HALO_ROWS * W  # 1904
QOFF = 3             # query rows sit at local rows 3..30

QH, QW = 14, 8       # query tile: 112 queries
NTI, NTJ = 2, 7      # tile grid per core
BR, BC = 20, 16      # padded key band (real cols: 14)
BAND = BR * BC       # 320
BR0 = {0: 0, 1: 14}                            # band local row start per ti
BC0 = [0, 5, 13, 21, 29, 37, 40]               # band local col start per tj
HCASE = [0, 1, 1, 1, 1, 1, 2]                  # tj -> horizontal case


def _win(p, n):
    """Clamped window start + bias row offset (matches reference _nbhd)."""
    start = np.clip(p - NS, 0, n - KS)
    ph = np.where(p < NS, KS - 1 - p, np.where(p + NS >= n, n - p - 1, NS))
    return start, ph


def _expb_tables(rpb, s):
    """exp(bias) * window-mask tables, [NTI, 3 hcases, NH, 112, BAND] f32.

    Key index within a band is k = r*BC + c (r: 20 band rows, c: 16 cols,
    last 2 cols are alignment padding -> always masked)."""
    out = np.zeros((NTI, 3, NH, 128, BAND), np.float32)
    qr = np.arange(QH)
    qc = np.arange(QW)
    r = np.arange(BR)
    c = np.arange(BC)
    for ti in range(NTI):
        g_qr = 28 * s + 14 * ti + qr                  # global query rows
        g_br = 28 * s - 3 + BR0[ti] + r               # global band rows
        vh_start, vh_ph = _win(g_qr, H)
        for hc, tj in ((0, 0), (1, 1), (2, 6)):
            g_qc = 8 * tj + qc
            g_bc = BC0[tj] + c
            vw_start, vw_ph = _win(g_qc, W)
            # valid[qr, r] / bias row index
            okr = (g_br[None, :] >= vh_start[:, None]) & (
                g_br[None, :] <= vh_start[:, None] + KS - 1)
            bi = vh_ph[:, None] + (g_br[None, :] - vh_start[:, None])
            okc = (g_bc[None, :] >= vw_start[:, None]) & (
                g_bc[None, :] <= vw_start[:, None] + KS - 1)
            bj = vw_ph[:, None] + (g_bc[None, :] - vw_start[:, None])
            bi = np.clip(bi, 0, 2 * KS - 2)
            bj = np.clip(bj, 0, 2 * KS - 2)
            for h in range(NH):
                bias = rpb[h][bi[:, None, :, None], bj[None, :, None, :]]
                val = np.exp(bias) * (okr[:, None, :, None] &
                                      okc[None, :, None, :])
                # [qr, qc, r, c] -> [q, k]
                out[ti, hc, h, :QH * QW] = val.reshape(QH * QW, BAND)
    return out


def _build_bass():
    import concourse.bass as bass
    import concourse.mybir as mybir
    import concourse.tile as tile
    from concourse import bacc

    fp32 = mybir.dt.float32
    bf16 = mybir.dt.bfloat16
    AF = mybir.ActivationFunctionType
    ALU = mybir.AluOpType

    nc = bacc.Bacc("TRN2", target_bir_lowering=False, debug=False)
    xT_d = nc.declare_dram_parameter("xT", [DIM, TOK], bf16, isOutput=False)
    wq_d = nc.declare_dram_parameter("wq", [DIM, DIM], bf16, isOutput=False)
    wk_d = nc.declare_dram_parameter("wk", [DIM, DIM], bf16, isOutput=False)
    wv_d = nc.declare_dram_parameter("wv", [DIM, DIM], bf16, isOutput=False)
    wp_d = nc.declare_dram_parameter("wp", [DIM, DIM], bf16, isOutput=False)
    bq_d = nc.declare_dram_parameter("bq", [DIM, 1], fp32, isOutput=False)
    eb_d = nc.declare_dram_parameter(
        "expB", [NTI * 3 * NH, 128, BAND], bf16, isOutput=False)
    out_d = nc.declare_dram_parameter(
        "out", [NTI * NTJ, QH * QW, DIM], fp32, isOutput=True)
    vnat_d = nc.dram_tensor("v_nat", [TOK + 4 * W, DIM], bf16)

    with tile.TileContext(nc) as tc:
        with tc.tile_pool(name="const", bufs=1) as cpool:
            wq = cpool.tile([DIM, DIM], bf16, tag="wq")
            wk = cpool.tile([DIM, DIM], bf16, tag="wk")
            wv = cpool.tile([DIM, DIM], bf16, tag="wv")
            wp = cpool.tile([DIM, DIM], bf16, tag="wp")
            bq = cpool.tile([DIM, 1], fp32, tag="bq")
            xT = cpool.tile([DIM, TOK], bf16, tag="xT")
            # qT is tile-major, 128-wide per tile (cols 112-127 zero pad
            # so score matmuls run M=128 with FWL weight loads)
            qT = cpool.tile([DIM, NTI * NTJ * 128], bf16, tag="qT")
            kT = cpool.tile([DIM, TOK], bf16, tag="kT")
            eb = cpool.tile([128, NTI * 3 * NH * BAND], bf16, tag="eb")
            for t, d in ((wq, wq_d), (wk, wk_d), (wv, wv_d), (wp, wp_d),
                         (bq, bq_d)):
                nc.sync.dma_start(t[:], d[:])
            for ci in range(4):
                sl = bass.ts(ci, TOK // 4)
                nc.sync.dma_start(xT[:, sl], xT_d[:, sl])
            # per-case loads so early tiles don't gate on the full table
            for cs in range(NTI * 3):
                nc.sync.dma_start(
                    eb[:, cs * NH * BAND:(cs + 1) * NH * BAND].rearrange(
                        "q (t k) -> q t k", t=NH),
                    eb_d[cs * NH:(cs + 1) * NH].rearrange(
                        "t q k -> q t k"))

            # ---- Stage A: qT / kT projections + v_nat to DRAM ----
            # bias goes through an ACT-local copy so consumers only need
            # same-engine FIFO ordering (instruction wait-slot limit).
            bq2 = cpool.tile([DIM, 1], fp32, tag="bq2")
            nc.scalar.copy(bq2[:], bq[:])
            nc.gpsimd.memset(
                qT[:].rearrange("p (t x) -> p t x", x=128)[:, :, QH * QW:],
                0.0)
            NCHUNK = 4
            CH = TOK // NCHUNK  # 476
            with (
                tc.tile_pool(name="psA", bufs=4, space="PSUM") as psA,
                tc.tile_pool(name="sbA", bufs=3) as sbA,
            ):
                for ci in range(NCHUNK):
                    sl = bass.ts(ci, CH)
                    pk = psA.tile([DIM, CH], fp32, tag="pq")
                    nc.tensor.matmul(pk[:], wk[:], xT[:, sl])
                    nc.scalar.copy(kT[:, sl], pk[:])
                for ci in range(4):  # q: half a tile-row (7 rows) per chunk
                    ti = ci // 2
                    r0 = QOFF + 14 * ti + 7 * (ci % 2)
                    pq = psA.tile([DIM, 7 * W], fp32, tag="pq")
                    nc.tensor.matmul(pq[:], wq[:],
                                     xT[:, r0 * W:r0 * W + 7 * W])
                    # escape reorders row-major (r, tj, qc) -> tile-major
                    qsl = qT[:, :].rearrange(
                        "p (t qr qc) -> p t qr qc", qr=16, qc=QW)[
                        :, NTJ * ti:NTJ * (ti + 1),
                        7 * (ci % 2):7 * (ci % 2) + 7, :]
                    pqv = pq[:, :].rearrange(
                        "p (r tj qc) -> p tj r qc", r=7, qc=QW)
                    nc.scalar.activation(qsl, pqv, AF.Identity,
                                         bias=bq2[:, 0:1])
                # v natural [tok, C]: PE -> PSUM -> DVE cast -> SBUF -> DRAM
                vsb = cpool.tile([128, 15 * DIM], bf16, tag="vsb")
                zpad = cpool.tile([128, 4 * W * DIM // 128], bf16,
                                  tag="zpad")
                nc.gpsimd.memset(zpad[:], 0.0)
                # tokens 1904-1919 of the last vsb chunk are junk: zero the
                # tail partitions (32-aligned start; real rows rewritten).
                nc.gpsimd.memset(vsb[96:, bass.ts(14, DIM)], 0.0)
                for ci in range(15):
                    t0 = ci * 128
                    tn = min(128, TOK - t0)
                    pv = psA.tile([128, DIM], fp32, tag="pv")
                    nc.tensor.matmul(pv[:tn, :], xT[:, t0:t0 + tn], wv[:])
                    nc.vector.tensor_copy(vsb[:tn, bass.ts(ci, DIM)],
                                          pv[:tn, :])
                nc.sync.dma_start(
                    vnat_d[:15 * 128, :].rearrange(
                        "(c p) d -> p c d", p=128),
                    vsb[:].rearrange("p (c d) -> p c d", d=DIM))
                nc.sync.dma_start(vnat_d[TOK:, :], zpad[:])


            # ---- Stage B: attention tiles ----
            with (
                tc.tile_pool(name="sb", bufs=10) as sb,
                tc.tile_pool(name="psS", bufs=3, space="PSUM") as psS,
                tc.tile_pool(name="psAV", bufs=1, space="PSUM") as psAV,
                tc.tile_pool(name="psP", bufs=1, space="PSUM") as psP,
            ):
                kT3 = kT[:].rearrange("p (r c) -> p r c", c=W)
                vnat3 = vnat_d[:].rearrange("(r c) d -> r c d", c=W)
                kTb = cpool.tile([128, NTI * NTJ * 384], bf16, tag="kTb")
                for t in range(NTI * NTJ):
                    ti, tj = divmod(t, NTJ)
                    nc.vector.tensor_copy(
                        kTb[:, t * 384:t * 384 + BAND].rearrange(
                            "p (r c) -> p r c", c=BC),
                        kT3[:, BR0[ti]:BR0[ti] + BR,
                            BC0[tj]:BC0[tj] + BC])
                posb = cpool.tile([QH * QW, NTI * NTJ * DIM], fp32,
                                  tag="posb")

                # Software-pipelined by one tile: tile t's scores/exp/mask/
                # transpose (front) are emitted before tile t-1's AV/proj
                # (back), so the PE's in-order stream covers the ~1.3us
                # XBAR-transpose latency with the next tile's matmuls.
                def emit_front(t):
                    ti, tj = divmod(t, NTJ)
                    br0, bc0 = BR0[ti], BC0[tj]
                    case = (ti * 3 + HCASE[tj]) * NH
                    # v band gather; layout per 128-key chunk:
                    # [V_h0 |1| V_h1 |1| V_h2 |1| V_h3 |1] (33 cols each);
                    # ones columns ride the AV matmul as softmax sums.
                    vband = sb.tile([128, 3 * 132], bf16, tag="vband")
                    nc.gpsimd.memset(
                        vband[:].rearrange(
                            "p (x u) -> p x u", u=33)[:, :, HD:], 1.0)
                    for j, rn in ((0, 8), (1, 8), (2, 4)):
                        nc.sync.dma_start(
                            vband[:rn * BC, :].rearrange(
                                "p (c h d) -> p c h d", c=3, h=NH)[
                                :, j, :, :HD],
                            vnat3[br0 + 8 * j:br0 + 8 * j + rn,
                                  bc0:bc0 + BC, :].rearrange(
                                "r c (h d) -> r c h d", h=NH))
                    P = sb.tile([128, 4 * 3 * DIM], bf16, tag="P")
                    nc.gpsimd.memset(
                        P[:].rearrange("p (x u) -> p x u", u=3 * DIM)[
                            :, :, BAND:], 0.0)
                    for hp in range(2):
                        S = psS.tile([128, 1024], fp32, tag="S")
                        for i in range(2):
                            h = 2 * hp + i
                            nc.tensor.matmul(
                                S[:, 512 * i:512 * i + BAND],
                                qT[32 * h:32 * h + 32,
                                   bass.ts(t, 128)],
                                kTb[32 * h:32 * h + 32,
                                    t * 384:t * 384 + BAND],
                                tile_position=(32 * h, 0))
                        expS = sb.tile([128, 2 * BAND], bf16,
                                       tag="expS")
                        nc.scalar.activation(
                            expS[:].rearrange("q (h k) -> q h k", h=2),
                            S[:].rearrange(
                                "q (h k) -> q h k", h=2)[:, :, :BAND],
                            AF.Exp)
                        for i in range(2):
                            h = 2 * hp + i
                            nc.vector.tensor_tensor(
                                P[:, 384 * h:384 * h + BAND],
                                expS[:, bass.ts(i, BAND)],
                                eb[:, (case + h) * BAND:
                                   (case + h + 1) * BAND],
                                ALU.mult)
                    PT = sb.tile([128, 4 * 3 * DIM], bf16, tag="PT")
                    nc.sync.dma_start(
                        PT[:].rearrange("p (j q) -> p j q", j=12),
                        P[:], transpose=True)
                    return vband, PT

                def emit_proj(t, attnT):
                    po = psP.tile([128, DIM], fp32, tag="po")
                    nc.tensor.matmul(po[:], attnT[:], wp[:])
                    nc.scalar.copy(posb[:, bass.ts(t, DIM)],
                                   po[:QH * QW, :])

                def emit_back(t, vband, PT):
                    attn = sb.tile([QH * QW, DIM + 16], bf16, tag="attn")
                    for hp in range(2):
                        rs = sb.tile([QH * QW, 2], fp32, tag="rs")
                        av = psAV.tile([128, 2 * 33], fp32, tag="av")
                        for i in range(2):
                            h = 2 * hp + i
                            for j, kn in ((0, 128), (1, 128), (2, 64)):
                                nc.tensor.matmul(
                                    av[:, bass.ts(i, 33)],
                                    PT[:kn, bass.ts(3 * h + j, 128)],
                                    vband[:kn, 132 * j + 33 * h:
                                          132 * j + 33 * (h + 1)],
                                    start=(j == 0), stop=(j == 2))
                        # col 32 of each av block = sum(exp) per query
                        nc.vector.reciprocal(
                            rs[:].rearrange("q (i u) -> q i u", u=1),
                            av[:QH * QW, :].rearrange(
                                "q (i d) -> q i d", i=2)[:, :, HD:HD + 1])
                        for i in range(2):
                            h = 2 * hp + i
                            nc.vector.tensor_scalar_mul(
                                attn[:, HD * h:HD * (h + 1)],
                                av[:QH * QW, 33 * i:33 * i + HD],
                                rs[:, i:i + 1])
                    attnT = sb.tile([128, 128], bf16, tag="attnT")
                    nc.vector.memset(attnT[:, QH * QW:], 0.0)
                    nc.sync.dma_start(attnT[:, :QH * QW], attn[:, :DIM],
                                      transpose=True)
                    return attnT
                NT = NTI * NTJ
                fronts, backs = {}, {}
                for t in range(NT + 2):
                    if t < NT:
                        fronts[t] = emit_front(t)
                    if 1 <= t <= NT:
                        backs[t - 1] = emit_back(t - 1, *fronts.pop(t - 1))
                    if t >= 2:
                        emit_proj(t - 2, backs.pop(t - 2))
                nc.sync.dma_start(
                    out_d[:].rearrange("t q d -> q t d"),
                    posb[:].rearrange("q (t d) -> q t d", d=DIM))
    nc.compile()
    return nc


_CACHE = {}


def _get_runner():
    """Compile the SPMD program once; returns (run_fn, bench_fn)."""
    if "runner" in _CACHE:
        return _CACHE["runner"]
    import jax
    from jax.experimental.shard_map import shard_map
    from jax.sharding import Mesh, NamedSharding, PartitionSpec
    import concourse.mybir as mybir
    from concourse import bass2jax

    bass2jax.install_neuronx_cc_hook()
    nc = _build_bass()
    n_cores = 8

    in_names, out_names, out_avals = [], [], []
    for alloc in nc.m.functions[0].allocations:
        if not isinstance(alloc, mybir.MemoryLocationSet):
            continue
        name = alloc.memorylocations[0].name
        if alloc.kind == "ExternalInput":
            in_names.append(name)
        elif alloc.kind == "ExternalOutput":
            out_names.append(name)
            out_avals.append(jax.core.ShapedArray(
                tuple(alloc.tensor_shape), mybir.dt.np(alloc.dtype)))
    part_name = (nc.partition_id_tensor.name
                 if nc.partition_id_tensor else None)
    if part_name in in_names:
        in_names.remove(part_name)
    n_params = len(in_names)
    all_names = in_names + out_names
    if part_name is not None:
        all_names = all_names + [part_name]

    def _body(*args):
        operands = list(args)
        if part_name is not None:
            operands.append(bass2jax.partition_id_tensor())
        return tuple(bass2jax._bass_exec_p.bind(
            *operands, out_avals=tuple(out_avals), in_names=tuple(all_names),
            out_names=tuple(out_names), lowering_input_output_aliases=(),
            sim_require_finite=True, sim_require_nnan=True, nc=nc))

    devices = jax.devices()[:n_cores]
    mesh = Mesh(np.asarray(devices), ("core",))
    spec = PartitionSpec("core")
    n_args = n_params + len(out_names)
    sharded = jax.jit(
        shard_map(_body, mesh=mesh, in_specs=(spec,) * n_args,
                  out_specs=(spec,) * len(out_names), check_rep=False),
        keep_unused=True)
    sharding = NamedSharding(mesh, spec)

    def _put(in_maps):
        arrs = []
        for i, name in enumerate(in_names):
            arrs.append(jax.device_put(np.concatenate(
                [in_maps[c][name] for c in range(n_cores)], axis=0),
                sharding))
        for av in out_avals:
            arrs.append(jax.device_put(
                np.zeros((n_cores * av.shape[0], *av.shape[1:]), av.dtype),
                sharding))
        return arrs

    def run_fn(in_maps):
        arrs = _put(in_maps)
        outs = sharded(*arrs)
        return [
            {name: np.asarray(outs[i]).reshape(n_cores, *out_avals[i].shape)[c]
             for i, name in enumerate(out_names)}
            for c in range(n_cores)
        ]

    def bench_fn(in_maps, iters=6, burst=8):
        # Amortized per-execution time: queue `burst` async dispatches and
        # block once; slope vs a single dispatch removes the host/axon
        # round-trip constant.
        import time
        arrs = _put(in_maps)
        jax.block_until_ready(sharded(*arrs))
        b1 = bn = float("inf")
        for _ in range(iters):
            t0 = time.perf_counter()
            jax.block_until_ready(sharded(*arrs))
            b1 = min(b1, time.perf_counter() - t0)
            t0 = time.perf_counter()
            outs = [sharded(*arrs) for _ in range(burst)]
            jax.block_until_ready(outs)
            bn = min(bn, time.perf_counter() - t0)
        per = (bn - b1) / (burst - 1)
        return max(per, 0.0) * 1e9
    _CACHE["runner"] = (run_fn, bench_fn)
    return _CACHE["runner"]


def _make_in_maps(x, w_qkv, b_qkv, rpb, w_proj, b_proj):
    x = np.asarray(x, np.float32)
    w_qkv = np.asarray(w_qkv, np.float32)
    b_qkv = np.asarray(b_qkv, np.float32)
    rpb = np.asarray(rpb, np.float32)
    w_proj = np.asarray(w_proj, np.float32)
    b_proj = np.asarray(b_proj, np.float32)

    wq = (w_qkv[:, 0:128] * SCALE).astype(ml_dtypes.bfloat16)
    wk = w_qkv[:, 128:256].astype(ml_dtypes.bfloat16)
    wv = w_qkv[:, 256:384].astype(ml_dtypes.bfloat16)
    wp = w_proj.astype(ml_dtypes.bfloat16)
    bq = (b_qkv[0:128] * SCALE).astype(np.float32).reshape(128, 1)
    # k-bias: softmax-invariant, dropped. v-bias folds into output bias.
    out_bias = b_proj + b_qkv[256:384] @ w_proj

    in_maps = []
    for core in range(8):
        b, s = divmod(core, 2)
        lo = 28 * s - 3
        rows = np.clip(np.arange(lo, lo + HALO_ROWS), 0, H - 1)
        xs = x[b, rows, :, :]                       # [34, 56, 128]
        if s == 0:
            xs[0:3] = 0.0
        else:
            xs[31:34] = 0.0
        xT = np.ascontiguousarray(
            xs.reshape(TOK, DIM).T).astype(ml_dtypes.bfloat16)
        expB = _expb_tables(rpb, s).reshape(
            NTI * 3 * NH, 128, BAND).astype(ml_dtypes.bfloat16)
        in_maps.append(dict(xT=xT, wq=wq, wk=wk, wv=wv, wp=wp, bq=bq,
                            expB=expB))
    return in_maps, out_bias


def _gather(results, out_bias):
    out = np.zeros((B, H, W, DIM), np.float32)
    for core in range(8):
        b, s = divmod(core, 2)
        o = results[core]["out"].reshape(NTI, NTJ, QH, QW, DIM)
        o = o.transpose(0, 2, 1, 3, 4).reshape(SLAB, W, DIM)
        out[b, 28 * s:28 * (s + 1)] = o
    out += out_bias
    return out


def kernel(x, w_qkv, b_qkv, rpb, w_proj, b_proj):
    in_maps, out_bias = _make_in_maps(x, w_qkv, b_qkv, rpb, w_proj, b_proj)
    run_fn, _ = _get_runner()
    return _gather(run_fn(in_maps), out_bias)


def bench(x, w_qkv, b_qkv, rpb, w_proj, b_proj, iters=20):
    """Returns (output, best_wall_ns) using the cached compiled runner."""
    in_maps, out_bias = _make_in_maps(x, w_qkv, b_qkv, rpb, w_proj, b_proj)
    run_fn, bench_fn = _get_runner()
    out = _gather(run_fn(in_maps), out_bias)
    return out, bench_fn(in_maps, iters)
